# revision 3
# baseline (speedup 1.0000x reference)
"""Trainium2 Bass kernel for nn_Encoder_67173288509869 (sparse_attention).

Computes, for each batch b (one NeuronCore per batch, 8 cores):
  scores[b]  = tril(g * (0.4*exp(-d^2/(2 l^2)) + 0.3*exp(-d/l)), -1)   [L, L]
  hidden[b]  = [sin(arc+phi) | cos(arc+phi) | type_emb[event_type]]    [L, 544]
  t_diff[b]  = |t_j - t_i|                                             [L, L]

Key structure exploited: l and g depend on (i, j) only through
(event_type[i], event_type[j]) — 21 distinct values each — so
r = 1/(l+eps) and g are 21x21 tables computed on host from the (tiny)
parameters and expanded on-device with exact one-hot matmuls on the
TensorEngine.  The positional part sin/cos(arc+phi) uses host-side
range-reduced arc tables (ACT Sin is accurate to |x| <= ~3.3).
"""

import sys

sys.path.insert(0, "/opt/trn_rl_repo")

import math
from threading import Lock

import numpy as np

import concourse.bacc as bacc
import concourse.mybir as mybir
from concourse.tile import TileContext, add_dep_helper
from concourse.bass_utils import run_bass_kernel_spmd

F32 = mybir.dt.float32
U32 = mybir.dt.uint32
AF = mybir.ActivationFunctionType
ALU = mybir.AluOpType

B, L = 8, 2048
D_MODEL = 512
D_HALF = D_MODEL // 2          # 256
D_TYPE = 32
NT = 21                        # num_types + padding idx
REG = 5.0
BETAS = (0.4, 0.3, 1.0)
EPS = 1e-6

P = 128                        # partition dim
NB = L // P                    # 16 row blocks
CHUNK = 1024                   # column chunk for the main elementwise pipe

LN_B0 = float(np.log(BETAS[0] * BETAS[2]))   # folded into exp(kse) bias
LN_B1 = float(np.log(BETAS[1] * BETAS[2]))   # folded into exp(kex) bias

_lock = Lock()
_cache = {}


def _build_nc():
    nc = bacc.Bacc("TRN2", target_bir_lowering=False, debug=False, num_devices=8)

    # Float biases for activation() must exist as const APs.
    for val in (LN_B0, LN_B1):
        t = nc.alloc_sbuf_tensor(f"const-f32-{val}", [P, 1], F32)
        nc.gpsimd.memset(t.ap(), val)
        nc.const_aps.aps[(F32, val)] = t.ap()
    nc.all_engine_barrier()

    # Per-core inputs (batch-sharded data + replicated constants).
    t_in = nc.dram_tensor("t_in", [L], F32, kind="ExternalInput")
    et_in = nc.dram_tensor("et_in", [L], F32, kind="ExternalInput")
    wt_in = nc.dram_tensor("wt_in", [D_HALF], F32, kind="ExternalInput")
    arcs_in = nc.dram_tensor("arcs_in", [L, D_HALF], F32, kind="ExternalInput")
    arcc_in = nc.dram_tensor("arcc_in", [L, D_HALF], F32, kind="ExternalInput")
    rtab_in = nc.dram_tensor("rtab_in", [NT, NT], F32, kind="ExternalInput")
    gtab_in = nc.dram_tensor("gtab_in", [NT, NT], F32, kind="ExternalInput")
    etab_in = nc.dram_tensor("etab_in", [NT, D_TYPE], F32, kind="ExternalInput")
    iota_in = nc.dram_tensor("iota_in", [NT, 1], F32, kind="ExternalInput")
    tri_in = nc.dram_tensor("tri_in", [P, P], F32, kind="ExternalInput")

    scores_o = nc.dram_tensor("scores_o", [L, L], F32, kind="ExternalOutput")
    hidden_o = nc.dram_tensor("hidden_o", [L, D_MODEL + D_TYPE], F32,
                              kind="ExternalOutput")
    tdiff_o = nc.dram_tensor("tdiff_o", [L, L], F32, kind="ExternalOutput")

    last_sin = [None]
    first_main_act = [None]

    with TileContext(nc) as tc:
        with tc.tile_pool(name="const", bufs=1) as cp:
            # --- setup: broadcasts + one-hots + table row-gathers ---
            t_b = cp.tile([P, L], F32)
            nc.sync.dma_start(t_b[:], t_in[:].partition_broadcast(P))
            t_cols = cp.tile([P, NB], F32)
            nc.sync.dma_start(t_cols[:], t_in[:].rearrange("(n p) -> p n", p=P))
            wt_b = cp.tile([P, D_HALF], F32)
            nc.sync.dma_start(wt_b[:], wt_in[:].partition_broadcast(P))
            et_b = cp.tile([NT, L], F32)
            nc.sync.dma_start(et_b[:], et_in[:].partition_broadcast(NT))
            iov = cp.tile([NT, 1], F32)
            nc.sync.dma_start(iov[:], iota_in[:])
            r_sb = cp.tile([NT, NT], F32)
            nc.sync.dma_start(r_sb[:], rtab_in[:])
            g_sb = cp.tile([NT, NT], F32)
            nc.sync.dma_start(g_sb[:], gtab_in[:])
            e_sb = cp.tile([NT, D_TYPE], F32)
            nc.sync.dma_start(e_sb[:], etab_in[:])
            tri_sb = cp.tile([P, P], F32)
            nc.sync.dma_start(tri_sb[:], tri_in[:])

            oht = cp.tile([NT, L], F32)   # one-hot^T: oht[k, j] = (et[j] == k)
            nc.vector.tensor_scalar(oht[:], et_b[:], iov[:], None, ALU.is_equal)

            # W_r[t, i] = Rtab[et[i], t]; W_g likewise (lhsT for the expand mm)
            w_r = cp.tile([NT, L], F32)
            w_g = cp.tile([NT, L], F32)
            with tc.tile_pool(name="psA", bufs=2, space="PSUM") as psA:
                for ib in range(NB):
                    i0 = ib * P
                    wr_ps = psA.tile([NT, P], F32, name=f"wr_ps{ib}", tag="wr_ps")
                    nc.tensor.matmul(wr_ps[:], r_sb[:], oht[:, i0:i0 + P],
                                     start=True, stop=True)
                    nc.vector.tensor_copy(w_r[:, i0:i0 + P], wr_ps[:])
                    wg_ps = psA.tile([NT, P], F32, name=f"wg_ps{ib}", tag="wg_ps")
                    nc.tensor.matmul(wg_ps[:], g_sb[:], oht[:, i0:i0 + P],
                                     start=True, stop=True)
                    nc.vector.tensor_copy(w_g[:, i0:i0 + P], wg_ps[:])

                # --- hidden phase (uses ACT Sin; runs before any Exp) ---
                with tc.tile_pool(name="hidp", bufs=3) as hp:
                    for ib in range(NB):
                        i0 = ib * P
                        t_col = t_cols[:, ib:ib + 1]
                        arcs_t = hp.tile([P, D_HALF], F32, name=f"arcs_t{ib}",
                                         tag="arcs_t")
                        nc.sync.dma_start(arcs_t[:], arcs_in[i0:i0 + P, :])
                        arcc_t = hp.tile([P, D_HALF], F32, name=f"arcc_t{ib}",
                                         tag="arcc_t")
                        nc.sync.dma_start(arcc_t[:], arcc_in[i0:i0 + P, :])
                        hid = hp.tile([P, D_MODEL + D_TYPE], F32,
                                      name=f"hid{ib}", tag="hid")
                        x1 = hp.tile([P, D_HALF], F32, name=f"x1_{ib}", tag="x1")
                        nc.vector.scalar_tensor_tensor(
                            x1[:], wt_b[:], t_col, arcs_t[:], ALU.mult, ALU.add)
                        nc.scalar.activation(hid[:, 0:D_HALF], x1[:], AF.Sin)
                        x2 = hp.tile([P, D_HALF], F32, name=f"x2_{ib}", tag="x2")
                        nc.vector.scalar_tensor_tensor(
                            x2[:], wt_b[:], t_col, arcc_t[:], ALU.mult, ALU.add)
                        si = nc.scalar.activation(
                            hid[:, D_HALF:D_MODEL], x2[:], AF.Sin)
                        last_sin[0] = si

                        te_ps = psA.tile([P, D_TYPE], F32, name=f"te_ps{ib}",
                                         tag="te_ps")
                        nc.tensor.matmul(te_ps[:], oht[:, i0:i0 + P], e_sb[:],
                                         start=True, stop=True)
                        nc.vector.tensor_copy(
                            hid[:, D_MODEL:D_MODEL + D_TYPE], te_ps[:])
                        nc.sync.dma_start(hidden_o[i0:i0 + P, :], hid[:])

            # --- main phase: t_diff + scores ---
            with tc.tile_pool(name="mainp", bufs=2) as mp, \
                 tc.tile_pool(name="psB", bufs=2, space="PSUM") as psB:
                for ib in range(NB):
                    i0 = ib * P
                    w = i0 + P            # scores computed for j < w
                    t_col = t_cols[:, ib:ib + 1]

                    d = mp.tile([P, L], F32, name=f"d{ib}", tag="d")
                    nc.vector.tensor_scalar(d[:], t_b[:], t_col, None,
                                            ALU.subtract)
                    du = d[:].bitcast(U32)
                    nc.vector.tensor_scalar(du, du, 0x7FFFFFFF, None,
                                            ALU.bitwise_and)
                    nc.sync.dma_start(tdiff_o[i0:i0 + P, :], d[:])

                    for jc in range(0, w, CHUNK):
                        cw = min(CHUNK, w - jc)
                        r_ps = psB.tile([P, CHUNK], F32, name=f"r_ps{ib}_{jc}",
                                        tag="r_ps")
                        g_ps = psB.tile([P, CHUNK], F32, name=f"g_ps{ib}_{jc}",
                                        tag="g_ps")
                        for cs in range(0, cw, 512):
                            cl = min(512, cw - cs)
                            nc.tensor.matmul(
                                r_ps[:, cs:cs + cl], w_r[:, i0:i0 + P],
                                oht[:, jc + cs:jc + cs + cl],
                                start=True, stop=True)
                            nc.tensor.matmul(
                                g_ps[:, cs:cs + cl], w_g[:, i0:i0 + P],
                                oht[:, jc + cs:jc + cs + cl],
                                start=True, stop=True)

                        q = mp.tile([P, CHUNK], F32, name=f"q{ib}_{jc}", tag="q")
                        nc.vector.tensor_tensor(
                            q[:, :cw], d[:, jc:jc + cw], r_ps[:, :cw], ALU.mult)
                        q2 = mp.tile([P, CHUNK], F32, name=f"q2_{ib}_{jc}",
                                     tag="q2")
                        sq = nc.scalar.activation(q2[:, :cw], q[:, :cw],
                                                  AF.Square)
                        if first_main_act[0] is None:
                            first_main_act[0] = sq
                        kse = mp.tile([P, CHUNK], F32, name=f"kse{ib}_{jc}",
                                      tag="kse")
                        nc.scalar.activation(kse[:, :cw], q2[:, :cw], AF.Exp,
                                             bias=LN_B0, scale=-0.5)
                        kex = mp.tile([P, CHUNK], F32, name=f"kex{ib}_{jc}",
                                      tag="kex")
                        nc.scalar.activation(kex[:, :cw], q[:, :cw], AF.Exp,
                                             bias=LN_B1, scale=-1.0)
                        s = mp.tile([P, CHUNK], F32, name=f"s{ib}_{jc}", tag="s")
                        nc.gpsimd.tensor_tensor(
                            s[:, :cw], kse[:, :cw], kex[:, :cw], ALU.add)
                        out = mp.tile([P, CHUNK], F32, name=f"out{ib}_{jc}",
                                      tag="out")
                        nc.vector.tensor_tensor(
                            out[:, :cw], s[:, :cw], g_ps[:, :cw], ALU.mult)
                        if jc <= i0 < jc + cw:
                            dd = i0 - jc   # diagonal block: strict lower mask
                            nc.gpsimd.tensor_tensor(
                                out[:, dd:dd + P], out[:, dd:dd + P],
                                tri_sb[:], ALU.mult)
                        nc.sync.dma_start(scores_o[i0:i0 + P, jc:jc + cw],
                                          out[:, :cw])
                    # columns >= w stay zero: output buffers are pre-zeroed
                    # by the runner (donated zero buffers / zeroed out_maps).

    if first_main_act[0] is not None and last_sin[0] is not None:
        fi = getattr(first_main_act[0], "ins", first_main_act[0])
        li = getattr(last_sin[0], "ins", last_sin[0])
        add_dep_helper(fi, li, False, "group Sin before Exp (ACT table sets)")

    nc.compile()
    return nc


def _host_prep(event_time, event_type, Wt_pos, type_emb, w_l, b_l, w_g, b_g):
    """Host-side parameter-table + positional-constant preparation."""
    f32 = np.float32
    # 21x21 tables (parameter-only): r = 1/(softplus(pa_j+pb_i+b_l)+eps), g
    te64 = type_emb.astype(np.float64)
    ew_pa = te64 @ w_l[:D_TYPE].astype(np.float64)   # j side
    ew_pb = te64 @ w_l[D_TYPE:].astype(np.float64)   # i side
    ew_ga = te64 @ w_g[:D_TYPE].astype(np.float64)
    ew_gb = te64 @ w_g[D_TYPE:].astype(np.float64)
    xl = ew_pa[None, :] + ew_pb[:, None] + float(b_l)
    l64 = np.log1p(np.exp(xl)) + EPS
    rtab = (1.0 / l64).astype(f32)
    xg = REG * (ew_ga[None, :] + ew_gb[:, None] + float(b_g))
    gtab = (1.0 / (1.0 + np.exp(-xg))).astype(f32)

    # positional arc tables, range-reduced to [-pi, pi)
    factor = f32(-math.log(10000.0) / D_MODEL)
    div_term = np.exp(np.arange(0, D_MODEL, 2).astype(f32) * factor).astype(f32)
    pos = np.arange(L).astype(f32)
    arc32 = (pos[:, None] * div_term[None, :]).astype(f32)
    a64 = arc32.astype(np.float64)
    arcs = (np.mod(a64 + np.pi, 2 * np.pi) - np.pi).astype(f32)
    arcc = (np.mod(a64 + np.pi / 2 + np.pi, 2 * np.pi) - np.pi).astype(f32)

    iota = np.arange(NT, dtype=f32).reshape(NT, 1)
    tri = np.tril(np.ones((P, P), dtype=f32), -1)

    in_maps = []
    for b in range(B):
        in_maps.append(dict(
            t_in=np.ascontiguousarray(event_time[b].astype(f32)),
            et_in=np.ascontiguousarray(event_type[b].astype(f32)),
            wt_in=np.ascontiguousarray(Wt_pos.astype(f32)),
            arcs_in=arcs, arcc_in=arcc,
            rtab_in=rtab, gtab_in=gtab,
            etab_in=np.ascontiguousarray(type_emb.astype(f32)),
            iota_in=iota, tri_in=tri,
        ))
    return in_maps


def get_nc():
    with _lock:
        if "nc" not in _cache:
            _cache["nc"] = _build_nc()
    return _cache["nc"]


def kernel(event_type, event_time, arrival_times=None, Wt_pos=None,
           type_emb=None, w_l=None, b_l=None, w_g=None, b_g=None):
    event_type = np.asarray(event_type)
    event_time = np.asarray(event_time, dtype=np.float32)
    Wt_pos = np.asarray(Wt_pos, dtype=np.float32)
    type_emb = np.asarray(type_emb, dtype=np.float32)
    w_l = np.asarray(w_l, dtype=np.float32)
    w_g = np.asarray(w_g, dtype=np.float32)
    b_l = float(np.asarray(b_l))
    b_g = float(np.asarray(b_g))

    nc = get_nc()
    in_maps = _host_prep(event_time, event_type, Wt_pos, type_emb,
                         w_l, b_l, w_g, b_g)
    res = run_bass_kernel_spmd(nc, in_maps, core_ids=list(range(B)))

    scores = np.stack([res.results[b]["scores_o"] for b in range(B)])
    hidden = np.stack([res.results[b]["hidden_o"] for b in range(B)])
    tdiff = np.stack([res.results[b]["tdiff_o"] for b in range(B)])
    return scores, hidden, tdiff


# revision 4
# speedup vs baseline: 8.9790x; 8.9790x over previous
"""Trainium2 Bass kernel for nn_Encoder_67173288509869 (sparse_attention).

Computes, for each batch b (one NeuronCore per batch, 8 cores):
  scores[b]  = tril(g * (0.4*exp(-d^2/(2 l^2)) + 0.3*exp(-d/l)), -1)   [L, L]
  hidden[b]  = [sin(arc+phi) | cos(arc+phi) | type_emb[event_type]]    [L, 544]
  t_diff[b]  = |t_j - t_i|                                             [L, L]

Key structure exploited: l and g depend on (i, j) only through
(event_type[i], event_type[j]) — 21 distinct values each — so
r = 1/(l+eps) and g are 21x21 tables computed on host from the (tiny)
parameters and expanded on-device with exact one-hot matmuls on the
TensorEngine.  The positional part sin/cos(arc+phi) uses host-side
range-reduced arc tables (ACT Sin is accurate to |x| <= ~3.3).
"""

import sys

sys.path.insert(0, "/opt/trn_rl_repo")

import math
from threading import Lock

import numpy as np

import concourse.bacc as bacc
import concourse.mybir as mybir
from concourse.tile import TileContext, add_dep_helper
from concourse.bass_utils import run_bass_kernel_spmd

F32 = mybir.dt.float32
U32 = mybir.dt.uint32
AF = mybir.ActivationFunctionType
ALU = mybir.AluOpType

B, L = 8, 2048
D_MODEL = 512
D_HALF = D_MODEL // 2          # 256
D_TYPE = 32
NT = 21                        # num_types + padding idx
REG = 5.0
BETAS = (0.4, 0.3, 1.0)
EPS = 1e-6

P = 128                        # partition dim
NB = L // P                    # 16 row blocks
CHUNK = 1024                   # column chunk for the main elementwise pipe

LN_B0 = float(np.log(BETAS[0] * BETAS[2]))   # folded into exp(kse) bias
LN_B1 = float(np.log(BETAS[1] * BETAS[2]))   # folded into exp(kex) bias

_lock = Lock()
_cache = {}


def _build_nc(repeat: int = 1):
    nc = bacc.Bacc("TRN2", target_bir_lowering=False, debug=False, num_devices=8)

    # Float biases for activation() must exist as const APs.
    for val in (LN_B0, LN_B1):
        t = nc.alloc_sbuf_tensor(f"const-f32-{val}", [P, 1], F32)
        nc.gpsimd.memset(t.ap(), val)
        nc.const_aps.aps[(F32, val)] = t.ap()
    nc.all_engine_barrier()

    # Per-core inputs (batch-sharded data + replicated constants).
    t_in = nc.dram_tensor("t_in", [L], F32, kind="ExternalInput")
    et_in = nc.dram_tensor("et_in", [L], F32, kind="ExternalInput")
    wt_in = nc.dram_tensor("wt_in", [D_HALF], F32, kind="ExternalInput")
    arcs_in = nc.dram_tensor("arcs_in", [L, D_HALF], F32, kind="ExternalInput")
    arcc_in = nc.dram_tensor("arcc_in", [L, D_HALF], F32, kind="ExternalInput")
    rtab_in = nc.dram_tensor("rtab_in", [NT, NT], F32, kind="ExternalInput")
    gtab_in = nc.dram_tensor("gtab_in", [NT, NT], F32, kind="ExternalInput")
    etab_in = nc.dram_tensor("etab_in", [NT, D_TYPE], F32, kind="ExternalInput")
    iota_in = nc.dram_tensor("iota_in", [NT, 1], F32, kind="ExternalInput")
    tri_in = nc.dram_tensor("tri_in", [P, P], F32, kind="ExternalInput")

    scores_o = nc.dram_tensor("scores_o", [L, L], F32, kind="ExternalOutput")
    hidden_o = nc.dram_tensor("hidden_o", [L, D_MODEL + D_TYPE], F32,
                              kind="ExternalOutput")
    tdiff_o = nc.dram_tensor("tdiff_o", [L, L], F32, kind="ExternalOutput")

    last_sin = [None]
    first_main_act = [None]

    with TileContext(nc) as tc:
        with tc.tile_pool(name="const", bufs=1) as cp:
            # --- setup: broadcasts + one-hots + table row-gathers ---
            t_b = cp.tile([P, L], F32)
            nc.sync.dma_start(t_b[:], t_in[:].partition_broadcast(P))
            t_cols = cp.tile([P, NB], F32)
            nc.sync.dma_start(t_cols[:], t_in[:].rearrange("(n p) -> p n", p=P))
            wt_b = cp.tile([P, D_HALF], F32)
            nc.sync.dma_start(wt_b[:], wt_in[:].partition_broadcast(P))
            et_b = cp.tile([NT, L], F32)
            nc.sync.dma_start(et_b[:], et_in[:].partition_broadcast(NT))
            iov = cp.tile([NT, 1], F32)
            nc.sync.dma_start(iov[:], iota_in[:])
            r_sb = cp.tile([NT, NT], F32)
            nc.sync.dma_start(r_sb[:], rtab_in[:])
            g_sb = cp.tile([NT, NT], F32)
            nc.sync.dma_start(g_sb[:], gtab_in[:])
            e_sb = cp.tile([NT, D_TYPE], F32)
            nc.sync.dma_start(e_sb[:], etab_in[:])
            tri_sb = cp.tile([P, P], F32)
            nc.sync.dma_start(tri_sb[:], tri_in[:])

            oht = cp.tile([NT, L], F32)   # one-hot^T: oht[k, j] = (et[j] == k)
            nc.vector.tensor_scalar(oht[:], et_b[:], iov[:], None, ALU.is_equal)

            # W_r[t, i] = Rtab[et[i], t]; W_g likewise (lhsT for the expand mm)
            w_r = cp.tile([NT, L], F32)
            w_g = cp.tile([NT, L], F32)
            with tc.tile_pool(name="psA", bufs=2, space="PSUM") as psA:
                for ib in range(NB):
                    i0 = ib * P
                    wr_ps = psA.tile([NT, P], F32, name=f"wr_ps{ib}", tag="wr_ps")
                    nc.tensor.matmul(wr_ps[:], r_sb[:], oht[:, i0:i0 + P],
                                     start=True, stop=True)
                    nc.vector.tensor_copy(w_r[:, i0:i0 + P], wr_ps[:])
                    wg_ps = psA.tile([NT, P], F32, name=f"wg_ps{ib}", tag="wg_ps")
                    nc.tensor.matmul(wg_ps[:], g_sb[:], oht[:, i0:i0 + P],
                                     start=True, stop=True)
                    nc.vector.tensor_copy(w_g[:, i0:i0 + P], wg_ps[:])

                # --- hidden phase (uses ACT Sin; runs before any Exp) ---
                with tc.tile_pool(name="hidp", bufs=3) as hp:
                    for ib in range(NB):
                        i0 = ib * P
                        t_col = t_cols[:, ib:ib + 1]
                        arcs_t = hp.tile([P, D_HALF], F32, name=f"arcs_t{ib}",
                                         tag="arcs_t")
                        nc.sync.dma_start(arcs_t[:], arcs_in[i0:i0 + P, :])
                        arcc_t = hp.tile([P, D_HALF], F32, name=f"arcc_t{ib}",
                                         tag="arcc_t")
                        nc.sync.dma_start(arcc_t[:], arcc_in[i0:i0 + P, :])
                        hid = hp.tile([P, D_MODEL + D_TYPE], F32,
                                      name=f"hid{ib}", tag="hid")
                        x1 = hp.tile([P, D_HALF], F32, name=f"x1_{ib}", tag="x1")
                        nc.vector.scalar_tensor_tensor(
                            x1[:], wt_b[:], t_col, arcs_t[:], ALU.mult, ALU.add)
                        nc.scalar.activation(hid[:, 0:D_HALF], x1[:], AF.Sin)
                        x2 = hp.tile([P, D_HALF], F32, name=f"x2_{ib}", tag="x2")
                        nc.vector.scalar_tensor_tensor(
                            x2[:], wt_b[:], t_col, arcc_t[:], ALU.mult, ALU.add)
                        si = nc.scalar.activation(
                            hid[:, D_HALF:D_MODEL], x2[:], AF.Sin)
                        last_sin[0] = si

                        te_ps = psA.tile([P, D_TYPE], F32, name=f"te_ps{ib}",
                                         tag="te_ps")
                        nc.tensor.matmul(te_ps[:], oht[:, i0:i0 + P], e_sb[:],
                                         start=True, stop=True)
                        nc.vector.tensor_copy(
                            hid[:, D_MODEL:D_MODEL + D_TYPE], te_ps[:])
                        nc.sync.dma_start(hidden_o[i0:i0 + P, :], hid[:])

            # --- main phase: t_diff + scores ---
            with tc.tile_pool(name="mainp", bufs=2) as mp, \
                 tc.tile_pool(name="psB", bufs=2, space="PSUM") as psB:
                for rep, ib in [(r, i) for r in range(repeat)
                                for i in range(NB)]:
                    i0 = ib * P
                    w = i0 + P            # scores computed for j < w
                    t_col = t_cols[:, ib:ib + 1]

                    d = mp.tile([P, L], F32, name=f"d{ib}_r{rep}", tag="d")
                    nc.vector.tensor_scalar(d[:], t_b[:], t_col, None,
                                            ALU.subtract)
                    du = d[:].bitcast(U32)
                    nc.vector.tensor_scalar(du, du, 0x7FFFFFFF, None,
                                            ALU.bitwise_and)
                    nc.sync.dma_start(tdiff_o[i0:i0 + P, :], d[:])

                    for jc in range(0, w, CHUNK):
                        cw = min(CHUNK, w - jc)
                        r_ps = psB.tile([P, CHUNK], F32, name=f"r_ps{ib}_{jc}_r{rep}",
                                        tag="r_ps")
                        g_ps = psB.tile([P, CHUNK], F32, name=f"g_ps{ib}_{jc}_r{rep}",
                                        tag="g_ps")
                        for cs in range(0, cw, 512):
                            cl = min(512, cw - cs)
                            nc.tensor.matmul(
                                r_ps[:, cs:cs + cl], w_r[:, i0:i0 + P],
                                oht[:, jc + cs:jc + cs + cl],
                                start=True, stop=True)
                            nc.tensor.matmul(
                                g_ps[:, cs:cs + cl], w_g[:, i0:i0 + P],
                                oht[:, jc + cs:jc + cs + cl],
                                start=True, stop=True)

                        q = mp.tile([P, CHUNK], F32, name=f"q{ib}_{jc}_r{rep}", tag="q")
                        nc.vector.tensor_tensor(
                            q[:, :cw], d[:, jc:jc + cw], r_ps[:, :cw], ALU.mult)
                        q2 = mp.tile([P, CHUNK], F32, name=f"q2_{ib}_{jc}_r{rep}",
                                     tag="q2")
                        sq = nc.scalar.activation(q2[:, :cw], q[:, :cw],
                                                  AF.Square)
                        if first_main_act[0] is None:
                            first_main_act[0] = sq
                        kse = mp.tile([P, CHUNK], F32, name=f"kse{ib}_{jc}_r{rep}",
                                      tag="kse")
                        nc.scalar.activation(kse[:, :cw], q2[:, :cw], AF.Exp,
                                             bias=LN_B0, scale=-0.5)
                        kex = mp.tile([P, CHUNK], F32, name=f"kex{ib}_{jc}_r{rep}",
                                      tag="kex")
                        nc.scalar.activation(kex[:, :cw], q[:, :cw], AF.Exp,
                                             bias=LN_B1, scale=-1.0)
                        s = mp.tile([P, CHUNK], F32, name=f"s{ib}_{jc}_r{rep}", tag="s")
                        nc.gpsimd.tensor_tensor(
                            s[:, :cw], kse[:, :cw], kex[:, :cw], ALU.add)
                        out = mp.tile([P, CHUNK], F32, name=f"out{ib}_{jc}_r{rep}",
                                      tag="out")
                        nc.vector.tensor_tensor(
                            out[:, :cw], s[:, :cw], g_ps[:, :cw], ALU.mult)
                        if jc <= i0 < jc + cw:
                            dd = i0 - jc   # diagonal block: strict lower mask
                            nc.gpsimd.tensor_tensor(
                                out[:, dd:dd + P], out[:, dd:dd + P],
                                tri_sb[:], ALU.mult)
                        nc.sync.dma_start(scores_o[i0:i0 + P, jc:jc + cw],
                                          out[:, :cw])
                    # columns >= w stay zero: output buffers are pre-zeroed
                    # by the runner (donated zero buffers / zeroed out_maps).

    if first_main_act[0] is not None and last_sin[0] is not None:
        fi = getattr(first_main_act[0], "ins", first_main_act[0])
        li = getattr(last_sin[0], "ins", last_sin[0])
        add_dep_helper(fi, li, False, "group Sin before Exp (ACT table sets)")

    nc.compile()
    return nc


def _host_prep(event_time, event_type, Wt_pos, type_emb, w_l, b_l, w_g, b_g):
    """Host-side parameter-table + positional-constant preparation."""
    f32 = np.float32
    # 21x21 tables (parameter-only): r = 1/(softplus(pa_j+pb_i+b_l)+eps), g
    te64 = type_emb.astype(np.float64)
    ew_pa = te64 @ w_l[:D_TYPE].astype(np.float64)   # j side
    ew_pb = te64 @ w_l[D_TYPE:].astype(np.float64)   # i side
    ew_ga = te64 @ w_g[:D_TYPE].astype(np.float64)
    ew_gb = te64 @ w_g[D_TYPE:].astype(np.float64)
    xl = ew_pa[None, :] + ew_pb[:, None] + float(b_l)
    l64 = np.log1p(np.exp(xl)) + EPS
    rtab = (1.0 / l64).astype(f32)
    xg = REG * (ew_ga[None, :] + ew_gb[:, None] + float(b_g))
    gtab = (1.0 / (1.0 + np.exp(-xg))).astype(f32)

    # positional arc tables, range-reduced to [-pi, pi)
    factor = f32(-math.log(10000.0) / D_MODEL)
    div_term = np.exp(np.arange(0, D_MODEL, 2).astype(f32) * factor).astype(f32)
    pos = np.arange(L).astype(f32)
    arc32 = (pos[:, None] * div_term[None, :]).astype(f32)
    a64 = arc32.astype(np.float64)
    arcs = (np.mod(a64 + np.pi, 2 * np.pi) - np.pi).astype(f32)
    arcc = (np.mod(a64 + np.pi / 2 + np.pi, 2 * np.pi) - np.pi).astype(f32)

    iota = np.arange(NT, dtype=f32).reshape(NT, 1)
    tri = np.tril(np.ones((P, P), dtype=f32), -1)

    in_maps = []
    for b in range(B):
        in_maps.append(dict(
            t_in=np.ascontiguousarray(event_time[b].astype(f32)),
            et_in=np.ascontiguousarray(event_type[b].astype(f32)),
            wt_in=np.ascontiguousarray(Wt_pos.astype(f32)),
            arcs_in=arcs, arcc_in=arcc,
            rtab_in=rtab, gtab_in=gtab,
            etab_in=np.ascontiguousarray(type_emb.astype(f32)),
            iota_in=iota, tri_in=tri,
        ))
    return in_maps


def get_nc(repeat: int = 1):
    with _lock:
        if ("nc", repeat) not in _cache:
            _cache[("nc", repeat)] = _build_nc(repeat)
    return _cache[("nc", repeat)]


def kernel(event_type, event_time, arrival_times=None, Wt_pos=None,
           type_emb=None, w_l=None, b_l=None, w_g=None, b_g=None):
    event_type = np.asarray(event_type)
    event_time = np.asarray(event_time, dtype=np.float32)
    Wt_pos = np.asarray(Wt_pos, dtype=np.float32)
    type_emb = np.asarray(type_emb, dtype=np.float32)
    w_l = np.asarray(w_l, dtype=np.float32)
    w_g = np.asarray(w_g, dtype=np.float32)
    b_l = float(np.asarray(b_l))
    b_g = float(np.asarray(b_g))

    nc = get_nc()
    in_maps = _host_prep(event_time, event_type, Wt_pos, type_emb,
                         w_l, b_l, w_g, b_g)
    res = run_bass_kernel_spmd(nc, in_maps, core_ids=list(range(B)))

    scores = np.stack([res.results[b]["scores_o"] for b in range(B)])
    hidden = np.stack([res.results[b]["hidden_o"] for b in range(B)])
    tdiff = np.stack([res.results[b]["tdiff_o"] for b in range(B)])
    return scores, hidden, tdiff


# revision 7
# speedup vs baseline: 10.5117x; 1.1707x over previous
"""Trainium2 Bass kernel for nn_Encoder_67173288509869 (sparse_attention).

Computes, for each batch b (one NeuronCore per batch, 8 cores):
  scores[b]  = tril(g * (0.4*exp(-d^2/(2 l^2)) + 0.3*exp(-d/l)), -1)   [L, L]
  hidden[b]  = [sin(arc+phi) | cos(arc+phi) | type_emb[event_type]]    [L, 544]
  t_diff[b]  = |t_j - t_i|                                             [L, L]

Key structure exploited: l and g depend on (i, j) only through
(event_type[i], event_type[j]) — 21 distinct values each — so
r = 1/(l+eps) and g are 21x21 tables computed on host from the (tiny)
parameters and expanded on-device with one-hot matmuls on the
TensorEngine.  Tables are split hi/lo into bf16 pairs and accumulated
in PSUM so the expansion runs at bf16 PE rate while reconstructing
fp32 values to ~1.6e-5.  The positional part sin/cos(arc+phi) uses
host-side range-reduced arc tables (ACT Sin is accurate to |x|<=~3.3).
"""

import sys

sys.path.insert(0, "/opt/trn_rl_repo")

import math
from threading import Lock

import ml_dtypes
import numpy as np

import concourse.bacc as bacc
import concourse.mybir as mybir
from concourse.tile import TileContext, add_dep_helper
from concourse.bass_utils import run_bass_kernel_spmd

F32 = mybir.dt.float32
BF16 = mybir.dt.bfloat16
U32 = mybir.dt.uint32
AF = mybir.ActivationFunctionType
ALU = mybir.AluOpType
BF16_NP = ml_dtypes.bfloat16

B, L = 8, 2048
D_MODEL = 512
D_HALF = D_MODEL // 2          # 256
D_TYPE = 32
NT = 21                        # num_types + padding idx
REG = 5.0
BETAS = (0.4, 0.3, 1.0)
EPS = 1e-6

P = 128                        # partition dim
NB = L // P                    # 16 row blocks
CHUNK = 1024                   # column chunk for the main elementwise pipe

LN_B0 = float(np.log(BETAS[0] * BETAS[2]))   # folded into exp(kse) bias
LN_B1 = float(np.log(BETAS[1] * BETAS[2]))   # folded into exp(kex) bias

_lock = Lock()
_cache = {}


def _build_nc(repeat: int = 1):
    nc = bacc.Bacc("TRN2", target_bir_lowering=False, debug=False, num_devices=8)

    # Float biases for activation() must exist as const APs.
    for val in (LN_B0, LN_B1):
        t = nc.alloc_sbuf_tensor(f"const-f32-{val}", [P, 1], F32)
        nc.gpsimd.memset(t.ap(), val)
        nc.const_aps.aps[(F32, val)] = t.ap()
    nc.all_engine_barrier()

    # Per-core inputs (batch-sharded data + replicated constants).
    t_in = nc.dram_tensor("t_in", [L], F32, kind="ExternalInput")
    et_in = nc.dram_tensor("et_in", [L], F32, kind="ExternalInput")
    wt_in = nc.dram_tensor("wt_in", [D_HALF], F32, kind="ExternalInput")
    arcs_in = nc.dram_tensor("arcs_in", [L, D_HALF], F32, kind="ExternalInput")
    arcc_in = nc.dram_tensor("arcc_in", [L, D_HALF], F32, kind="ExternalInput")
    rhi_in = nc.dram_tensor("rhi_in", [NT, NT], BF16, kind="ExternalInput")
    rlo_in = nc.dram_tensor("rlo_in", [NT, NT], BF16, kind="ExternalInput")
    ghi_in = nc.dram_tensor("ghi_in", [NT, NT], BF16, kind="ExternalInput")
    glo_in = nc.dram_tensor("glo_in", [NT, NT], BF16, kind="ExternalInput")
    ehi_in = nc.dram_tensor("ehi_in", [NT, D_TYPE], BF16, kind="ExternalInput")
    elo_in = nc.dram_tensor("elo_in", [NT, D_TYPE], BF16, kind="ExternalInput")
    iota_in = nc.dram_tensor("iota_in", [NT, 1], F32, kind="ExternalInput")
    tri_in = nc.dram_tensor("tri_in", [P, P], F32, kind="ExternalInput")

    scores_o = nc.dram_tensor("scores_o", [L, L], F32, kind="ExternalOutput")
    hidden_o = nc.dram_tensor("hidden_o", [L, D_MODEL + D_TYPE], F32,
                              kind="ExternalOutput")
    tdiff_o = nc.dram_tensor("tdiff_o", [L, L], F32, kind="ExternalOutput")

    last_sin = [None]
    first_main_exp = [None]

    with TileContext(nc) as tc:
        with tc.tile_pool(name="const", bufs=1) as cp:
            # --- setup: broadcasts + one-hots + table row-gathers ---
            t_b = cp.tile([P, L], F32)
            nc.sync.dma_start(t_b[:], t_in[:].partition_broadcast(P))
            t_cols = cp.tile([P, NB], F32)
            nc.sync.dma_start(t_cols[:], t_in[:].rearrange("(n p) -> p n", p=P))
            negt = cp.tile([P, NB], F32)
            nc.vector.tensor_scalar(negt[:], t_cols[:], -1.0, None, ALU.mult)
            wt_b = cp.tile([P, D_HALF], F32)
            nc.sync.dma_start(wt_b[:], wt_in[:].partition_broadcast(P))
            et_b = cp.tile([NT, L], F32)
            nc.sync.dma_start(et_b[:], et_in[:].partition_broadcast(NT))
            iov = cp.tile([NT, 1], F32)
            nc.sync.dma_start(iov[:], iota_in[:])
            rhi_sb = cp.tile([NT, NT], BF16)
            nc.sync.dma_start(rhi_sb[:], rhi_in[:])
            rlo_sb = cp.tile([NT, NT], BF16)
            nc.sync.dma_start(rlo_sb[:], rlo_in[:])
            ghi_sb = cp.tile([NT, NT], BF16)
            nc.sync.dma_start(ghi_sb[:], ghi_in[:])
            glo_sb = cp.tile([NT, NT], BF16)
            nc.sync.dma_start(glo_sb[:], glo_in[:])
            ehi_sb = cp.tile([NT, D_TYPE], BF16)
            nc.sync.dma_start(ehi_sb[:], ehi_in[:])
            elo_sb = cp.tile([NT, D_TYPE], BF16)
            nc.sync.dma_start(elo_sb[:], elo_in[:])
            tri_sb = cp.tile([P, P], F32)
            nc.sync.dma_start(tri_sb[:], tri_in[:])

            # one-hot^T in bf16 (exact 0/1): oht[k, j] = (et[j] == k)
            oht = cp.tile([NT, L], BF16)
            nc.vector.tensor_scalar(oht[:], et_b[:], iov[:], None, ALU.is_equal)

            # W tables: W_x[t, i] = Xtab[et[i], t]  (lhsT for the expand mm)
            w_tiles = {}
            with tc.tile_pool(name="psA", bufs=2, space="PSUM") as psA:
                for nm, tab in (("rhi", rhi_sb), ("rlo", rlo_sb),
                                ("ghi", ghi_sb), ("glo", glo_sb)):
                    wps = psA.tile([NT, L], F32, name=f"wps_{nm}", tag="wps", bufs=1)
                    for c in range(0, L, 512):
                        nc.tensor.matmul(wps[:, c:c + 512], tab[:],
                                         oht[:, c:c + 512],
                                         start=True, stop=True)
                    wsb = cp.tile([NT, L], BF16, name=f"w_{nm}")
                    nc.vector.tensor_copy(wsb[:], wps[:])
                    w_tiles[nm] = wsb

                # --- hidden phase (uses ACT Sin; runs before any Exp) ---
                with tc.tile_pool(name="hidp", bufs=3) as hp:
                    for ib in range(NB):
                        i0 = ib * P
                        t_col = t_cols[:, ib:ib + 1]
                        arcs_t = hp.tile([P, D_HALF], F32, name=f"arcs_t{ib}",
                                         tag="arcs_t")
                        nc.sync.dma_start(arcs_t[:], arcs_in[i0:i0 + P, :])
                        arcc_t = hp.tile([P, D_HALF], F32, name=f"arcc_t{ib}",
                                         tag="arcc_t")
                        nc.sync.dma_start(arcc_t[:], arcc_in[i0:i0 + P, :])
                        hid = hp.tile([P, D_MODEL + D_TYPE], F32,
                                      name=f"hid{ib}", tag="hid")
                        x1 = hp.tile([P, D_HALF], F32, name=f"x1_{ib}", tag="x1")
                        nc.vector.scalar_tensor_tensor(
                            x1[:], wt_b[:], t_col, arcs_t[:], ALU.mult, ALU.add)
                        nc.scalar.activation(hid[:, 0:D_HALF], x1[:], AF.Sin)
                        x2 = hp.tile([P, D_HALF], F32, name=f"x2_{ib}", tag="x2")
                        nc.vector.scalar_tensor_tensor(
                            x2[:], wt_b[:], t_col, arcc_t[:], ALU.mult, ALU.add)
                        si = nc.scalar.activation(
                            hid[:, D_HALF:D_MODEL], x2[:], AF.Sin)
                        last_sin[0] = si

                        te_ps = psA.tile([P, D_TYPE], F32, name=f"te_ps{ib}",
                                         tag="te_ps")
                        nc.tensor.matmul(te_ps[:], oht[:, i0:i0 + P],
                                         ehi_sb[:], start=True, stop=False)
                        nc.tensor.matmul(te_ps[:], oht[:, i0:i0 + P],
                                         elo_sb[:], start=False, stop=True)
                        nc.vector.tensor_copy(
                            hid[:, D_MODEL:D_MODEL + D_TYPE], te_ps[:])
                        nc.sync.dma_start(hidden_o[i0:i0 + P, :], hid[:])

            # --- main phase: t_diff + scores ---
            with tc.tile_pool(name="mainp", bufs=2) as mp, \
                 tc.tile_pool(name="psB", bufs=2, space="PSUM") as psB:
                for rep, ib in [(r, i) for r in range(repeat)
                                for i in range(NB)]:
                    i0 = ib * P
                    w = i0 + P            # scores computed for j < w

                    d = mp.tile([P, L], F32, name=f"d{ib}_r{rep}", tag="d")
                    nc.scalar.activation(d[:], t_b[:], AF.Abs,
                                         bias=negt[:, ib:ib + 1], scale=1.0)
                    nc.sync.dma_start(tdiff_o[i0:i0 + P, :], d[:])

                    for jc in range(0, w, CHUNK):
                        cw = min(CHUNK, w - jc)
                        r_ps = psB.tile([P, CHUNK], F32,
                                        name=f"r_ps{ib}_{jc}_r{rep}", tag="r_ps")
                        g_ps = psB.tile([P, CHUNK], F32,
                                        name=f"g_ps{ib}_{jc}_r{rep}", tag="g_ps")
                        for cs in range(0, cw, 512):
                            cl = min(512, cw - cs)
                            co = jc + cs
                            nc.tensor.matmul(
                                r_ps[:, cs:cs + cl],
                                w_tiles["rhi"][:, i0:i0 + P],
                                oht[:, co:co + cl], start=True, stop=False)
                            nc.tensor.matmul(
                                r_ps[:, cs:cs + cl],
                                w_tiles["rlo"][:, i0:i0 + P],
                                oht[:, co:co + cl], start=False, stop=True)
                            nc.tensor.matmul(
                                g_ps[:, cs:cs + cl],
                                w_tiles["ghi"][:, i0:i0 + P],
                                oht[:, co:co + cl], start=True, stop=False)
                            nc.tensor.matmul(
                                g_ps[:, cs:cs + cl],
                                w_tiles["glo"][:, i0:i0 + P],
                                oht[:, co:co + cl], start=False, stop=True)

                        q = mp.tile([P, CHUNK], F32, name=f"q{ib}_{jc}_r{rep}",
                                    tag="q")
                        nc.vector.tensor_tensor(
                            q[:, :cw], d[:, jc:jc + cw], r_ps[:, :cw], ALU.mult)
                        q2 = mp.tile([P, CHUNK], F32,
                                     name=f"q2_{ib}_{jc}_r{rep}", tag="q2")
                        nc.gpsimd.tensor_tensor(q2[:, :cw], q[:, :cw],
                                                q[:, :cw], ALU.mult)
                        kse = mp.tile([P, CHUNK], F32,
                                      name=f"kse{ib}_{jc}_r{rep}", tag="kse")
                        ei = nc.scalar.activation(kse[:, :cw], q2[:, :cw],
                                                  AF.Exp, bias=LN_B0,
                                                  scale=-0.5)
                        if first_main_exp[0] is None:
                            first_main_exp[0] = ei
                        kex = mp.tile([P, CHUNK], F32,
                                      name=f"kex{ib}_{jc}_r{rep}", tag="kex")
                        nc.scalar.activation(kex[:, :cw], q[:, :cw], AF.Exp,
                                             bias=LN_B1, scale=-1.0)
                        s = mp.tile([P, CHUNK], F32, name=f"s{ib}_{jc}_r{rep}",
                                    tag="s")
                        nc.gpsimd.tensor_tensor(
                            s[:, :cw], kse[:, :cw], kex[:, :cw], ALU.add)
                        out = mp.tile([P, CHUNK], F32,
                                      name=f"out{ib}_{jc}_r{rep}", tag="out")
                        nc.vector.tensor_tensor(
                            out[:, :cw], s[:, :cw], g_ps[:, :cw], ALU.mult)
                        if jc <= i0 < jc + cw:
                            dd = i0 - jc   # diagonal block: strict lower mask
                            nc.gpsimd.tensor_tensor(
                                out[:, dd:dd + P], out[:, dd:dd + P],
                                tri_sb[:], ALU.mult)
                        nc.sync.dma_start(scores_o[i0:i0 + P, jc:jc + cw],
                                          out[:, :cw])
                    # columns >= w stay zero: output buffers are pre-zeroed
                    # by the runner (donated zero buffers / zeroed out_maps).

    if first_main_exp[0] is not None and last_sin[0] is not None:
        fi = getattr(first_main_exp[0], "ins", first_main_exp[0])
        li = getattr(last_sin[0], "ins", last_sin[0])
        add_dep_helper(fi, li, False, "group Sin before Exp (ACT table sets)")

    nc.compile()
    return nc


def _split_bf16(x):
    hi = x.astype(BF16_NP)
    lo = (x - hi.astype(np.float32)).astype(BF16_NP)
    return hi, lo


def _host_prep(event_time, event_type, Wt_pos, type_emb, w_l, b_l, w_g, b_g):
    """Host-side parameter-table + positional-constant preparation."""
    f32 = np.float32
    # 21x21 tables (parameter-only): r = 1/(softplus(pa_j+pb_i+b_l)+eps), g
    te64 = type_emb.astype(np.float64)
    ew_pa = te64 @ w_l[:D_TYPE].astype(np.float64)   # j side
    ew_pb = te64 @ w_l[D_TYPE:].astype(np.float64)   # i side
    ew_ga = te64 @ w_g[:D_TYPE].astype(np.float64)
    ew_gb = te64 @ w_g[D_TYPE:].astype(np.float64)
    xl = ew_pa[None, :] + ew_pb[:, None] + float(b_l)
    l64 = np.log1p(np.exp(xl)) + EPS
    rtab = (1.0 / l64).astype(f32)
    xg = REG * (ew_ga[None, :] + ew_gb[:, None] + float(b_g))
    gtab = (1.0 / (1.0 + np.exp(-xg))).astype(f32)
    rhi, rlo = _split_bf16(rtab)
    ghi, glo = _split_bf16(gtab)
    ehi, elo = _split_bf16(type_emb.astype(f32))

    # positional arc tables, range-reduced to [-pi, pi)
    factor = f32(-math.log(10000.0) / D_MODEL)
    div_term = np.exp(np.arange(0, D_MODEL, 2).astype(f32) * factor).astype(f32)
    pos = np.arange(L).astype(f32)
    arc32 = (pos[:, None] * div_term[None, :]).astype(f32)
    a64 = arc32.astype(np.float64)
    arcs = (np.mod(a64 + np.pi, 2 * np.pi) - np.pi).astype(f32)
    arcc = (np.mod(a64 + np.pi / 2 + np.pi, 2 * np.pi) - np.pi).astype(f32)

    iota = np.arange(NT, dtype=f32).reshape(NT, 1)
    tri = np.tril(np.ones((P, P), dtype=f32), -1)

    in_maps = []
    for b in range(B):
        in_maps.append(dict(
            t_in=np.ascontiguousarray(event_time[b].astype(f32)),
            et_in=np.ascontiguousarray(event_type[b].astype(f32)),
            wt_in=np.ascontiguousarray(Wt_pos.astype(f32)),
            arcs_in=arcs, arcc_in=arcc,
            rhi_in=rhi, rlo_in=rlo, ghi_in=ghi, glo_in=glo,
            ehi_in=ehi, elo_in=elo,
            iota_in=iota, tri_in=tri,
        ))
    return in_maps


def get_nc(repeat: int = 1):
    with _lock:
        if ("nc", repeat) not in _cache:
            _cache[("nc", repeat)] = _build_nc(repeat)
    return _cache[("nc", repeat)]


def kernel(event_type, event_time, arrival_times=None, Wt_pos=None,
           type_emb=None, w_l=None, b_l=None, w_g=None, b_g=None):
    event_type = np.asarray(event_type)
    event_time = np.asarray(event_time, dtype=np.float32)
    Wt_pos = np.asarray(Wt_pos, dtype=np.float32)
    type_emb = np.asarray(type_emb, dtype=np.float32)
    w_l = np.asarray(w_l, dtype=np.float32)
    w_g = np.asarray(w_g, dtype=np.float32)
    b_l = float(np.asarray(b_l))
    b_g = float(np.asarray(b_g))

    nc = get_nc()
    in_maps = _host_prep(event_time, event_type, Wt_pos, type_emb,
                         w_l, b_l, w_g, b_g)
    res = run_bass_kernel_spmd(nc, in_maps, core_ids=list(range(B)))

    scores = np.stack([res.results[b]["scores_o"] for b in range(B)])
    hidden = np.stack([res.results[b]["hidden_o"] for b in range(B)])
    tdiff = np.stack([res.results[b]["tdiff_o"] for b in range(B)])
    return scores, hidden, tdiff


# revision 9
# speedup vs baseline: 10.6457x; 1.0127x over previous
"""Trainium2 Bass kernel for nn_Encoder_67173288509869 (sparse_attention).

Computes, for each batch b (one NeuronCore per batch, 8 cores):
  scores[b]  = tril(g * (0.4*exp(-d^2/(2 l^2)) + 0.3*exp(-d/l)), -1)   [L, L]
  hidden[b]  = [sin(arc+phi) | cos(arc+phi) | type_emb[event_type]]    [L, 544]
  t_diff[b]  = |t_j - t_i|                                             [L, L]

Key structure exploited: l and g depend on (i, j) only through
(event_type[i], event_type[j]) — 21 distinct values each — so
r = 1/(l+eps) and g are 21x21 tables computed on host from the (tiny)
parameters and expanded on-device with one-hot matmuls on the
TensorEngine.  Tables are split hi/lo into bf16 pairs and accumulated
in PSUM so the expansion runs at bf16 PE rate while reconstructing
fp32 values to ~1.6e-5.  The positional part sin/cos(arc+phi) uses
host-side range-reduced arc tables (ACT Sin is accurate to |x|<=~3.3).
"""

import sys

sys.path.insert(0, "/opt/trn_rl_repo")

import math
from threading import Lock

import ml_dtypes
import numpy as np

import concourse.bacc as bacc
import concourse.mybir as mybir
from concourse.tile import TileContext, add_dep_helper
from concourse.bass_utils import run_bass_kernel_spmd

F32 = mybir.dt.float32
BF16 = mybir.dt.bfloat16
U32 = mybir.dt.uint32
AF = mybir.ActivationFunctionType
ALU = mybir.AluOpType
BF16_NP = ml_dtypes.bfloat16

B, L = 8, 2048
D_MODEL = 512
D_HALF = D_MODEL // 2          # 256
D_TYPE = 32
NT = 21                        # num_types + padding idx
REG = 5.0
BETAS = (0.4, 0.3, 1.0)
EPS = 1e-6

P = 128                        # partition dim
NB = L // P                    # 16 row blocks
CHUNK = 1024                   # column chunk for the main elementwise pipe

NT2 = 64                       # K-stacked (hi row 0, lo row 32, zero pad)

LN_B0 = float(np.log(BETAS[0] * BETAS[2]))   # folded into exp(kse) bias
LN_B1 = float(np.log(BETAS[1] * BETAS[2]))   # folded into exp(kex) bias

_lock = Lock()
_cache = {}


def _build_nc(repeat: int = 1):
    nc = bacc.Bacc("TRN2", target_bir_lowering=False, debug=False, num_devices=8)

    # Float biases for activation() must exist as const APs.
    for val in (LN_B0, LN_B1):
        t = nc.alloc_sbuf_tensor(f"const-f32-{val}", [P, 1], F32)
        nc.gpsimd.memset(t.ap(), val)
        nc.const_aps.aps[(F32, val)] = t.ap()
    nc.all_engine_barrier()

    # Per-core inputs (batch-sharded data + replicated constants).
    t_in = nc.dram_tensor("t_in", [L], F32, kind="ExternalInput")
    et_in = nc.dram_tensor("et_in", [L], F32, kind="ExternalInput")
    wt_in = nc.dram_tensor("wt_in", [D_HALF], F32, kind="ExternalInput")
    arcs_in = nc.dram_tensor("arcs_in", [L, D_HALF], F32, kind="ExternalInput")
    arcc_in = nc.dram_tensor("arcc_in", [L, D_HALF], F32, kind="ExternalInput")
    rcat_in = nc.dram_tensor("rcat_in", [NT, NT2], BF16, kind="ExternalInput")
    gcat_in = nc.dram_tensor("gcat_in", [NT, NT2], BF16, kind="ExternalInput")
    ecat_in = nc.dram_tensor("ecat_in", [NT2, D_TYPE], BF16,
                             kind="ExternalInput")
    iota_in = nc.dram_tensor("iota_in", [NT, 1], F32, kind="ExternalInput")
    tri_in = nc.dram_tensor("tri_in", [P, P], F32, kind="ExternalInput")

    scores_o = nc.dram_tensor("scores_o", [L, L], F32, kind="ExternalOutput")
    hidden_o = nc.dram_tensor("hidden_o", [L, D_MODEL + D_TYPE], F32,
                              kind="ExternalOutput")
    tdiff_o = nc.dram_tensor("tdiff_o", [L, L], F32, kind="ExternalOutput")

    last_sin = [None]
    first_main_exp = [None]

    with TileContext(nc) as tc:
        with tc.tile_pool(name="const", bufs=1) as cp:
            # --- setup: broadcasts + one-hots + table row-gathers ---
            t_b = cp.tile([P, L], F32)
            nc.sync.dma_start(t_b[:], t_in[:].partition_broadcast(P))
            t_cols = cp.tile([P, NB], F32)
            nc.sync.dma_start(t_cols[:], t_in[:].rearrange("(n p) -> p n", p=P))
            negt = cp.tile([P, NB], F32)
            nc.vector.tensor_scalar(negt[:], t_cols[:], -1.0, None, ALU.mult)
            wt_b = cp.tile([P, D_HALF], F32)
            nc.sync.dma_start(wt_b[:], wt_in[:].partition_broadcast(P))
            et_b = cp.tile([NT, L], F32)
            nc.sync.dma_start(et_b[:], et_in[:].partition_broadcast(NT))
            iov = cp.tile([NT, 1], F32)
            nc.sync.dma_start(iov[:], iota_in[:])
            rcat_sb = cp.tile([NT, NT2], BF16)
            nc.sync.dma_start(rcat_sb[:], rcat_in[:])
            gcat_sb = cp.tile([NT, NT2], BF16)
            nc.sync.dma_start(gcat_sb[:], gcat_in[:])
            ecat_sb = cp.tile([NT2, D_TYPE], BF16)
            nc.sync.dma_start(ecat_sb[:], ecat_in[:])
            tri_sb = cp.tile([P, P], F32)
            nc.sync.dma_start(tri_sb[:], tri_in[:])

            # one-hot^T in bf16 (exact 0/1), stacked twice on the K axis
            # (rows 0-20 and 32-52; engine partition offsets must be
            # 32-aligned) so hi+lo table pairs contract in one K=64 matmul.
            oht = cp.tile([NT2, L], BF16)
            nc.gpsimd.memset(oht[:], 0.0)
            nc.vector.tensor_scalar(oht[0:NT, :], et_b[:], iov[:], None,
                                    ALU.is_equal)
            nc.vector.tensor_scalar(oht[32:32 + NT, :], et_b[:], iov[:], None,
                                    ALU.is_equal)

            # W tables, K-stacked: W2_x[0:21,i] = Xhi[et[i],:]^T, rows 21:42
            # the lo part — lhsT for the one-matmul hi+lo expand.
            w_tiles = {}
            with tc.tile_pool(name="psA", bufs=2, space="PSUM") as psA:
                for nm, tab in (("r", rcat_sb), ("g", gcat_sb)):
                    wps = psA.tile([NT2, L], F32, name=f"wps_{nm}",
                                   tag="wps", bufs=1)
                    for c in range(0, L, 512):
                        nc.tensor.matmul(wps[:, c:c + 512], tab[:],
                                         oht[0:NT, c:c + 512],
                                         start=True, stop=True)
                    wsb = cp.tile([NT2, L], BF16, name=f"w_{nm}")
                    nc.vector.tensor_copy(wsb[:], wps[:])
                    w_tiles[nm] = wsb

                # --- hidden phase (uses ACT Sin; runs before any Exp) ---
                with tc.tile_pool(name="hidp", bufs=3) as hp:
                    for ib in range(NB):
                        i0 = ib * P
                        t_col = t_cols[:, ib:ib + 1]
                        arcs_t = hp.tile([P, D_HALF], F32, name=f"arcs_t{ib}",
                                         tag="arcs_t")
                        nc.sync.dma_start(arcs_t[:], arcs_in[i0:i0 + P, :])
                        arcc_t = hp.tile([P, D_HALF], F32, name=f"arcc_t{ib}",
                                         tag="arcc_t")
                        nc.sync.dma_start(arcc_t[:], arcc_in[i0:i0 + P, :])
                        hid = hp.tile([P, D_MODEL + D_TYPE], F32,
                                      name=f"hid{ib}", tag="hid")
                        x1 = hp.tile([P, D_HALF], F32, name=f"x1_{ib}", tag="x1")
                        nc.vector.scalar_tensor_tensor(
                            x1[:], wt_b[:], t_col, arcs_t[:], ALU.mult, ALU.add)
                        nc.scalar.activation(hid[:, 0:D_HALF], x1[:], AF.Sin)
                        x2 = hp.tile([P, D_HALF], F32, name=f"x2_{ib}", tag="x2")
                        nc.vector.scalar_tensor_tensor(
                            x2[:], wt_b[:], t_col, arcc_t[:], ALU.mult, ALU.add)
                        si = nc.scalar.activation(
                            hid[:, D_HALF:D_MODEL], x2[:], AF.Sin)
                        last_sin[0] = si

                        te_ps = psA.tile([P, D_TYPE], F32, name=f"te_ps{ib}",
                                         tag="te_ps")
                        nc.tensor.matmul(te_ps[:], oht[:, i0:i0 + P],
                                         ecat_sb[:], start=True, stop=True)
                        nc.vector.tensor_copy(
                            hid[:, D_MODEL:D_MODEL + D_TYPE], te_ps[:])
                        nc.sync.dma_start(hidden_o[i0:i0 + P, :], hid[:])

            # --- main phase: t_diff + scores ---
            with tc.tile_pool(name="mainp", bufs=2) as mp, \
                 tc.tile_pool(name="psB", bufs=2, space="PSUM") as psB:
                for rep, ib in [(r, i) for r in range(repeat)
                                for i in range(NB)]:
                    i0 = ib * P
                    w = i0 + P            # scores computed for j < w

                    d = mp.tile([P, L], F32, name=f"d{ib}_r{rep}", tag="d")
                    if ib % 8 < 5:        # 10/16 on ACT, 6/16 on DVE
                        nc.scalar.activation(d[:], t_b[:], AF.Abs,
                                             bias=negt[:, ib:ib + 1], scale=1.0)
                    else:
                        nc.vector.tensor_scalar(d[:], t_b[:],
                                                t_cols[:, ib:ib + 1], None,
                                                ALU.subtract)
                        du = d[:].bitcast(U32)
                        nc.vector.tensor_scalar(du, du, 0x7FFFFFFF, None,
                                                ALU.bitwise_and)
                    nc.sync.dma_start(tdiff_o[i0:i0 + P, :], d[:])

                    for jc in range(0, w, CHUNK):
                        cw = min(CHUNK, w - jc)
                        r_ps = psB.tile([P, CHUNK], F32,
                                        name=f"r_ps{ib}_{jc}_r{rep}", tag="r_ps")
                        g_ps = psB.tile([P, CHUNK], F32,
                                        name=f"g_ps{ib}_{jc}_r{rep}", tag="g_ps")
                        for cs in range(0, cw, 512):
                            cl = min(512, cw - cs)
                            co = jc + cs
                            nc.tensor.matmul(
                                r_ps[:, cs:cs + cl],
                                w_tiles["r"][:, i0:i0 + P],
                                oht[:, co:co + cl], start=True, stop=True)
                            nc.tensor.matmul(
                                g_ps[:, cs:cs + cl],
                                w_tiles["g"][:, i0:i0 + P],
                                oht[:, co:co + cl], start=True, stop=True)

                        q = mp.tile([P, CHUNK], F32, name=f"q{ib}_{jc}_r{rep}",
                                    tag="q")
                        nc.vector.tensor_tensor(
                            q[:, :cw], d[:, jc:jc + cw], r_ps[:, :cw], ALU.mult)
                        q2 = mp.tile([P, CHUNK], F32,
                                     name=f"q2_{ib}_{jc}_r{rep}", tag="q2")
                        nc.gpsimd.tensor_tensor(q2[:, :cw], q[:, :cw],
                                                q[:, :cw], ALU.mult)
                        kse = mp.tile([P, CHUNK], F32,
                                      name=f"kse{ib}_{jc}_r{rep}", tag="kse")
                        ei = nc.scalar.activation(kse[:, :cw], q2[:, :cw],
                                                  AF.Exp, bias=LN_B0,
                                                  scale=-0.5)
                        if first_main_exp[0] is None:
                            first_main_exp[0] = ei
                        kex = mp.tile([P, CHUNK], F32,
                                      name=f"kex{ib}_{jc}_r{rep}", tag="kex")
                        nc.scalar.activation(kex[:, :cw], q[:, :cw], AF.Exp,
                                             bias=LN_B1, scale=-1.0)
                        s = mp.tile([P, CHUNK], F32, name=f"s{ib}_{jc}_r{rep}",
                                    tag="s")
                        nc.gpsimd.tensor_tensor(
                            s[:, :cw], kse[:, :cw], kex[:, :cw], ALU.add)
                        out = mp.tile([P, CHUNK], F32,
                                      name=f"out{ib}_{jc}_r{rep}", tag="out")
                        nc.vector.tensor_tensor(
                            out[:, :cw], s[:, :cw], g_ps[:, :cw], ALU.mult)
                        if jc <= i0 < jc + cw:
                            dd = i0 - jc   # diagonal block: strict lower mask
                            nc.vector.tensor_tensor(
                                out[:, dd:dd + P], out[:, dd:dd + P],
                                tri_sb[:], ALU.mult)
                        nc.sync.dma_start(scores_o[i0:i0 + P, jc:jc + cw],
                                          out[:, :cw])
                    # columns >= w stay zero: output buffers are pre-zeroed
                    # by the runner (donated zero buffers / zeroed out_maps).

    if first_main_exp[0] is not None and last_sin[0] is not None:
        fi = getattr(first_main_exp[0], "ins", first_main_exp[0])
        li = getattr(last_sin[0], "ins", last_sin[0])
        add_dep_helper(fi, li, False, "group Sin before Exp (ACT table sets)")

    nc.compile()
    return nc


def _split_bf16(x):
    hi = x.astype(BF16_NP)
    lo = (x - hi.astype(np.float32)).astype(BF16_NP)
    return hi, lo


def _host_prep(event_time, event_type, Wt_pos, type_emb, w_l, b_l, w_g, b_g):
    """Host-side parameter-table + positional-constant preparation."""
    f32 = np.float32
    # 21x21 tables (parameter-only): r = 1/(softplus(pa_j+pb_i+b_l)+eps), g
    te64 = type_emb.astype(np.float64)
    ew_pa = te64 @ w_l[:D_TYPE].astype(np.float64)   # j side
    ew_pb = te64 @ w_l[D_TYPE:].astype(np.float64)   # i side
    ew_ga = te64 @ w_g[:D_TYPE].astype(np.float64)
    ew_gb = te64 @ w_g[D_TYPE:].astype(np.float64)
    xl = ew_pa[None, :] + ew_pb[:, None] + float(b_l)
    l64 = np.log1p(np.exp(xl)) + EPS
    rtab = (1.0 / l64).astype(f32)
    xg = REG * (ew_ga[None, :] + ew_gb[:, None] + float(b_g))
    gtab = (1.0 / (1.0 + np.exp(-xg))).astype(f32)
    rhi, rlo = _split_bf16(rtab)
    ghi, glo = _split_bf16(gtab)
    ehi, elo = _split_bf16(type_emb.astype(f32))
    rcat = np.zeros((NT, NT2), dtype=BF16_NP)          # [21, 64]
    rcat[:, 0:NT] = rhi
    rcat[:, 32:32 + NT] = rlo
    gcat = np.zeros((NT, NT2), dtype=BF16_NP)
    gcat[:, 0:NT] = ghi
    gcat[:, 32:32 + NT] = glo
    ecat = np.zeros((NT2, D_TYPE), dtype=BF16_NP)      # [64, 32]
    ecat[0:NT] = ehi
    ecat[32:32 + NT] = elo

    # positional arc tables, range-reduced to [-pi, pi)
    factor = f32(-math.log(10000.0) / D_MODEL)
    div_term = np.exp(np.arange(0, D_MODEL, 2).astype(f32) * factor).astype(f32)
    pos = np.arange(L).astype(f32)
    arc32 = (pos[:, None] * div_term[None, :]).astype(f32)
    a64 = arc32.astype(np.float64)
    arcs = (np.mod(a64 + np.pi, 2 * np.pi) - np.pi).astype(f32)
    arcc = (np.mod(a64 + np.pi / 2 + np.pi, 2 * np.pi) - np.pi).astype(f32)

    iota = np.arange(NT, dtype=f32).reshape(NT, 1)
    tri = np.tril(np.ones((P, P), dtype=f32), -1)

    in_maps = []
    for b in range(B):
        in_maps.append(dict(
            t_in=np.ascontiguousarray(event_time[b].astype(f32)),
            et_in=np.ascontiguousarray(event_type[b].astype(f32)),
            wt_in=np.ascontiguousarray(Wt_pos.astype(f32)),
            arcs_in=arcs, arcc_in=arcc,
            rcat_in=rcat, gcat_in=gcat, ecat_in=ecat,
            iota_in=iota, tri_in=tri,
        ))
    return in_maps


def get_nc(repeat: int = 1):
    with _lock:
        if ("nc", repeat) not in _cache:
            _cache[("nc", repeat)] = _build_nc(repeat)
    return _cache[("nc", repeat)]


def kernel(event_type, event_time, arrival_times=None, Wt_pos=None,
           type_emb=None, w_l=None, b_l=None, w_g=None, b_g=None):
    event_type = np.asarray(event_type)
    event_time = np.asarray(event_time, dtype=np.float32)
    Wt_pos = np.asarray(Wt_pos, dtype=np.float32)
    type_emb = np.asarray(type_emb, dtype=np.float32)
    w_l = np.asarray(w_l, dtype=np.float32)
    w_g = np.asarray(w_g, dtype=np.float32)
    b_l = float(np.asarray(b_l))
    b_g = float(np.asarray(b_g))

    nc = get_nc()
    in_maps = _host_prep(event_time, event_type, Wt_pos, type_emb,
                         w_l, b_l, w_g, b_g)
    res = run_bass_kernel_spmd(nc, in_maps, core_ids=list(range(B)))

    scores = np.stack([res.results[b]["scores_o"] for b in range(B)])
    hidden = np.stack([res.results[b]["hidden_o"] for b in range(B)])
    tdiff = np.stack([res.results[b]["tdiff_o"] for b in range(B)])
    return scores, hidden, tdiff


# revision 10
# speedup vs baseline: 11.4226x; 1.0730x over previous
"""Trainium2 Bass kernel for nn_Encoder_67173288509869 (sparse_attention).

Computes, for each batch b (one NeuronCore per batch, 8 cores):
  scores[b]  = tril(g * (0.4*exp(-d^2/(2 l^2)) + 0.3*exp(-d/l)), -1)   [L, L]
  hidden[b]  = [sin(arc+phi) | cos(arc+phi) | type_emb[event_type]]    [L, 544]
  t_diff[b]  = |t_j - t_i|                                             [L, L]

Key structure exploited: l and g depend on (i, j) only through
(event_type[i], event_type[j]) — 21 distinct values each — so
r = 1/(l+eps) and g are 21x21 tables computed on host from the (tiny)
parameters and expanded on-device with one-hot matmuls on the
TensorEngine.  Tables are split hi/lo into bf16 pairs and accumulated
in PSUM so the expansion runs at bf16 PE rate while reconstructing
fp32 values to ~1.6e-5.  The positional part sin/cos(arc+phi) uses
host-side range-reduced arc tables (ACT Sin is accurate to |x|<=~3.3).
"""

import sys

sys.path.insert(0, "/opt/trn_rl_repo")

import math
from threading import Lock

import ml_dtypes
import numpy as np

import concourse.bacc as bacc
import concourse.mybir as mybir
from concourse.tile import TileContext, add_dep_helper
from concourse.bass_utils import run_bass_kernel_spmd

F32 = mybir.dt.float32
BF16 = mybir.dt.bfloat16
U32 = mybir.dt.uint32
AF = mybir.ActivationFunctionType
ALU = mybir.AluOpType
BF16_NP = ml_dtypes.bfloat16

B, L = 8, 2048
D_MODEL = 512
D_HALF = D_MODEL // 2          # 256
D_TYPE = 32
NT = 21                        # num_types + padding idx
REG = 5.0
BETAS = (0.4, 0.3, 1.0)
EPS = 1e-6

P = 128                        # partition dim
NB = L // P                    # 16 row blocks
CHUNK = 1024                   # column chunk for the main elementwise pipe

NT2 = 64                       # K-stacked (hi row 0, lo row 32, zero pad)

LN_B0 = float(np.log(BETAS[0] * BETAS[2]))   # folded into exp(kse) bias
LN_B1 = float(np.log(BETAS[1] * BETAS[2]))   # folded into exp(kex) bias

_lock = Lock()
_cache = {}


def _build_nc(repeat: int = 1):
    nc = bacc.Bacc("TRN2", target_bir_lowering=False, debug=False, num_devices=8)

    # Float biases for activation() must exist as const APs.
    for val in (LN_B0, LN_B1):
        t = nc.alloc_sbuf_tensor(f"const-f32-{val}", [P, 1], F32)
        nc.gpsimd.memset(t.ap(), val)
        nc.const_aps.aps[(F32, val)] = t.ap()
    nc.all_engine_barrier()

    # Per-core inputs (batch-sharded data + replicated constants).
    t_in = nc.dram_tensor("t_in", [L], F32, kind="ExternalInput")
    et_in = nc.dram_tensor("et_in", [L], F32, kind="ExternalInput")
    wt_in = nc.dram_tensor("wt_in", [D_HALF], F32, kind="ExternalInput")
    arcs_in = nc.dram_tensor("arcs_in", [L, D_HALF], F32, kind="ExternalInput")
    arcc_in = nc.dram_tensor("arcc_in", [L, D_HALF], F32, kind="ExternalInput")
    rcat_in = nc.dram_tensor("rcat_in", [NT, NT2], BF16, kind="ExternalInput")
    gcat_in = nc.dram_tensor("gcat_in", [NT, NT2], BF16, kind="ExternalInput")
    ecat_in = nc.dram_tensor("ecat_in", [NT2, D_TYPE], BF16,
                             kind="ExternalInput")
    iota_in = nc.dram_tensor("iota_in", [NT, 1], F32, kind="ExternalInput")
    tri_in = nc.dram_tensor("tri_in", [P, P], F32, kind="ExternalInput")

    scores_o = nc.dram_tensor("scores_o", [L, L], F32, kind="ExternalOutput")
    hidden_o = nc.dram_tensor("hidden_o", [L, D_MODEL + D_TYPE], F32,
                              kind="ExternalOutput")
    tdiff_o = nc.dram_tensor("tdiff_o", [L, L], F32, kind="ExternalOutput")

    last_sin = [None]
    first_main_exp = [None]

    with TileContext(nc) as tc:
        with tc.tile_pool(name="const", bufs=1) as cp:
            # --- setup: broadcasts + one-hots + table row-gathers ---
            t_b = cp.tile([P, L], F32)
            nc.sync.dma_start(t_b[:], t_in[:].partition_broadcast(P))
            t_cols = cp.tile([P, NB], F32)
            nc.sync.dma_start(t_cols[:], t_in[:].rearrange("(n p) -> p n", p=P))
            negt = cp.tile([P, NB], F32)
            nc.vector.tensor_scalar(negt[:], t_cols[:], -1.0, None, ALU.mult)
            wt_b = cp.tile([P, D_HALF], F32)
            nc.sync.dma_start(wt_b[:], wt_in[:].partition_broadcast(P))
            et_b = cp.tile([NT, L], F32)
            nc.sync.dma_start(et_b[:], et_in[:].partition_broadcast(NT))
            iov = cp.tile([NT, 1], F32)
            nc.sync.dma_start(iov[:], iota_in[:])
            rcat_sb = cp.tile([NT, NT2], BF16)
            nc.sync.dma_start(rcat_sb[:], rcat_in[:])
            gcat_sb = cp.tile([NT, NT2], BF16)
            nc.sync.dma_start(gcat_sb[:], gcat_in[:])
            ecat_sb = cp.tile([NT2, D_TYPE], BF16)
            nc.sync.dma_start(ecat_sb[:], ecat_in[:])
            tri_sb = cp.tile([P, P], F32)
            nc.sync.dma_start(tri_sb[:], tri_in[:])

            # one-hot^T in bf16 (exact 0/1), stacked twice on the K axis
            # (rows 0-20 and 32-52; engine partition offsets must be
            # 32-aligned) so hi+lo table pairs contract in one K=64 matmul.
            oht = cp.tile([NT2, L], BF16)
            nc.gpsimd.memset(oht[:], 0.0)
            nc.vector.tensor_scalar(oht[0:NT, :], et_b[:], iov[:], None,
                                    ALU.is_equal)
            nc.vector.tensor_scalar(oht[32:32 + NT, :], et_b[:], iov[:], None,
                                    ALU.is_equal)

            # W tables, K-stacked: W2_x[0:21,i] = Xhi[et[i],:]^T, rows 21:42
            # the lo part — lhsT for the one-matmul hi+lo expand.
            w_tiles = {}
            with tc.tile_pool(name="psA", bufs=2, space="PSUM") as psA:
                for nm, tab in (("r", rcat_sb), ("g", gcat_sb)):
                    wps = psA.tile([NT2, L], F32, name=f"wps_{nm}",
                                   tag="wps", bufs=1)
                    for c in range(0, L, 512):
                        nc.tensor.matmul(wps[:, c:c + 512], tab[:],
                                         oht[0:NT, c:c + 512],
                                         start=True, stop=True)
                    wsb = cp.tile([NT2, L], BF16, name=f"w_{nm}")
                    nc.vector.tensor_copy(wsb[:], wps[:])
                    w_tiles[nm] = wsb

                # --- hidden phase (uses ACT Sin; runs before any Exp) ---
                with tc.tile_pool(name="hidp", bufs=3) as hp:
                    for ib in range(NB):
                        i0 = ib * P
                        t_col = t_cols[:, ib:ib + 1]
                        arcs_t = hp.tile([P, D_HALF], F32, name=f"arcs_t{ib}",
                                         tag="arcs_t")
                        nc.sync.dma_start(arcs_t[:], arcs_in[i0:i0 + P, :])
                        arcc_t = hp.tile([P, D_HALF], F32, name=f"arcc_t{ib}",
                                         tag="arcc_t")
                        nc.sync.dma_start(arcc_t[:], arcc_in[i0:i0 + P, :])
                        hid = hp.tile([P, D_MODEL + D_TYPE], F32,
                                      name=f"hid{ib}", tag="hid")
                        x1 = hp.tile([P, D_HALF], F32, name=f"x1_{ib}", tag="x1")
                        nc.vector.scalar_tensor_tensor(
                            x1[:], wt_b[:], t_col, arcs_t[:], ALU.mult, ALU.add)
                        nc.scalar.activation(hid[:, 0:D_HALF], x1[:], AF.Sin)
                        x2 = hp.tile([P, D_HALF], F32, name=f"x2_{ib}", tag="x2")
                        nc.vector.scalar_tensor_tensor(
                            x2[:], wt_b[:], t_col, arcc_t[:], ALU.mult, ALU.add)
                        si = nc.scalar.activation(
                            hid[:, D_HALF:D_MODEL], x2[:], AF.Sin)
                        last_sin[0] = si

                        te_ps = psA.tile([P, D_TYPE], F32, name=f"te_ps{ib}",
                                         tag="te_ps")
                        nc.tensor.matmul(te_ps[:], oht[:, i0:i0 + P],
                                         ecat_sb[:], start=True, stop=True)
                        nc.vector.tensor_copy(
                            hid[:, D_MODEL:D_MODEL + D_TYPE], te_ps[:])
                        nc.sync.dma_start(hidden_o[i0:i0 + P, :], hid[:])

            # --- main phase: t_diff + scores ---
            with tc.tile_pool(name="mainp", bufs=3) as mp, \
                 tc.tile_pool(name="psB", bufs=2, space="PSUM") as psB:
                for rep, ib in [(r, i) for r in range(repeat)
                                for i in range(NB)]:
                    i0 = ib * P
                    w = i0 + P            # scores computed for j < w

                    d = mp.tile([P, L], F32, name=f"d{ib}_r{rep}", tag="d")
                    nc.scalar.activation(d[:], t_b[:], AF.Abs,
                                         bias=negt[:, ib:ib + 1], scale=1.0)
                    nc.sync.dma_start(tdiff_o[i0:i0 + P, :], d[:])

                    for jc in range(0, w, CHUNK):
                        cw = min(CHUNK, w - jc)
                        r_ps = psB.tile([P, CHUNK], F32,
                                        name=f"r_ps{ib}_{jc}_r{rep}", tag="r_ps")
                        g_ps = psB.tile([P, CHUNK], F32,
                                        name=f"g_ps{ib}_{jc}_r{rep}", tag="g_ps")
                        for cs in range(0, cw, 512):
                            cl = min(512, cw - cs)
                            co = jc + cs
                            nc.tensor.matmul(
                                r_ps[:, cs:cs + cl],
                                w_tiles["r"][:, i0:i0 + P],
                                oht[:, co:co + cl], start=True, stop=True)
                            nc.tensor.matmul(
                                g_ps[:, cs:cs + cl],
                                w_tiles["g"][:, i0:i0 + P],
                                oht[:, co:co + cl], start=True, stop=True)

                        q = mp.tile([P, CHUNK], F32, name=f"q{ib}_{jc}_r{rep}",
                                    tag="q")
                        nc.vector.tensor_tensor(
                            q[:, :cw], d[:, jc:jc + cw], r_ps[:, :cw], ALU.mult)
                        q2 = mp.tile([P, CHUNK], F32,
                                     name=f"q2_{ib}_{jc}_r{rep}", tag="q2")
                        nc.vector.tensor_tensor(q2[:, :cw], q[:, :cw],
                                                q[:, :cw], ALU.mult)
                        kse = mp.tile([P, CHUNK], F32,
                                      name=f"kse{ib}_{jc}_r{rep}", tag="kse")
                        ei = nc.scalar.activation(kse[:, :cw], q2[:, :cw],
                                                  AF.Exp, bias=LN_B0,
                                                  scale=-0.5)
                        if first_main_exp[0] is None:
                            first_main_exp[0] = ei
                        kex = mp.tile([P, CHUNK], F32,
                                      name=f"kex{ib}_{jc}_r{rep}", tag="kex")
                        nc.scalar.activation(kex[:, :cw], q[:, :cw], AF.Exp,
                                             bias=LN_B1, scale=-1.0)
                        s = mp.tile([P, CHUNK], F32, name=f"s{ib}_{jc}_r{rep}",
                                    tag="s")
                        nc.gpsimd.tensor_tensor(
                            s[:, :cw], kse[:, :cw], kex[:, :cw], ALU.add)
                        out = mp.tile([P, CHUNK], F32,
                                      name=f"out{ib}_{jc}_r{rep}", tag="out")
                        nc.vector.tensor_tensor(
                            out[:, :cw], s[:, :cw], g_ps[:, :cw], ALU.mult)
                        if jc <= i0 < jc + cw:
                            dd = i0 - jc   # diagonal block: strict lower mask
                            nc.gpsimd.tensor_tensor(
                                out[:, dd:dd + P], out[:, dd:dd + P],
                                tri_sb[:], ALU.mult)
                        nc.sync.dma_start(scores_o[i0:i0 + P, jc:jc + cw],
                                          out[:, :cw])
                    # columns >= w stay zero: output buffers are pre-zeroed
                    # by the runner (donated zero buffers / zeroed out_maps).

    if first_main_exp[0] is not None and last_sin[0] is not None:
        fi = getattr(first_main_exp[0], "ins", first_main_exp[0])
        li = getattr(last_sin[0], "ins", last_sin[0])
        add_dep_helper(fi, li, False, "group Sin before Exp (ACT table sets)")

    nc.compile()
    return nc


def _split_bf16(x):
    hi = x.astype(BF16_NP)
    lo = (x - hi.astype(np.float32)).astype(BF16_NP)
    return hi, lo


def _host_prep(event_time, event_type, Wt_pos, type_emb, w_l, b_l, w_g, b_g):
    """Host-side parameter-table + positional-constant preparation."""
    f32 = np.float32
    # 21x21 tables (parameter-only): r = 1/(softplus(pa_j+pb_i+b_l)+eps), g
    te64 = type_emb.astype(np.float64)
    ew_pa = te64 @ w_l[:D_TYPE].astype(np.float64)   # j side
    ew_pb = te64 @ w_l[D_TYPE:].astype(np.float64)   # i side
    ew_ga = te64 @ w_g[:D_TYPE].astype(np.float64)
    ew_gb = te64 @ w_g[D_TYPE:].astype(np.float64)
    xl = ew_pa[None, :] + ew_pb[:, None] + float(b_l)
    l64 = np.log1p(np.exp(xl)) + EPS
    rtab = (1.0 / l64).astype(f32)
    xg = REG * (ew_ga[None, :] + ew_gb[:, None] + float(b_g))
    gtab = (1.0 / (1.0 + np.exp(-xg))).astype(f32)
    rhi, rlo = _split_bf16(rtab)
    ghi, glo = _split_bf16(gtab)
    ehi, elo = _split_bf16(type_emb.astype(f32))
    rcat = np.zeros((NT, NT2), dtype=BF16_NP)          # [21, 64]
    rcat[:, 0:NT] = rhi
    rcat[:, 32:32 + NT] = rlo
    gcat = np.zeros((NT, NT2), dtype=BF16_NP)
    gcat[:, 0:NT] = ghi
    gcat[:, 32:32 + NT] = glo
    ecat = np.zeros((NT2, D_TYPE), dtype=BF16_NP)      # [64, 32]
    ecat[0:NT] = ehi
    ecat[32:32 + NT] = elo

    # positional arc tables, range-reduced to [-pi, pi)
    factor = f32(-math.log(10000.0) / D_MODEL)
    div_term = np.exp(np.arange(0, D_MODEL, 2).astype(f32) * factor).astype(f32)
    pos = np.arange(L).astype(f32)
    arc32 = (pos[:, None] * div_term[None, :]).astype(f32)
    a64 = arc32.astype(np.float64)
    arcs = (np.mod(a64 + np.pi, 2 * np.pi) - np.pi).astype(f32)
    arcc = (np.mod(a64 + np.pi / 2 + np.pi, 2 * np.pi) - np.pi).astype(f32)

    iota = np.arange(NT, dtype=f32).reshape(NT, 1)
    tri = np.tril(np.ones((P, P), dtype=f32), -1)

    in_maps = []
    for b in range(B):
        in_maps.append(dict(
            t_in=np.ascontiguousarray(event_time[b].astype(f32)),
            et_in=np.ascontiguousarray(event_type[b].astype(f32)),
            wt_in=np.ascontiguousarray(Wt_pos.astype(f32)),
            arcs_in=arcs, arcc_in=arcc,
            rcat_in=rcat, gcat_in=gcat, ecat_in=ecat,
            iota_in=iota, tri_in=tri,
        ))
    return in_maps


def get_nc(repeat: int = 1):
    with _lock:
        if ("nc", repeat) not in _cache:
            _cache[("nc", repeat)] = _build_nc(repeat)
    return _cache[("nc", repeat)]


def kernel(event_type, event_time, arrival_times=None, Wt_pos=None,
           type_emb=None, w_l=None, b_l=None, w_g=None, b_g=None):
    event_type = np.asarray(event_type)
    event_time = np.asarray(event_time, dtype=np.float32)
    Wt_pos = np.asarray(Wt_pos, dtype=np.float32)
    type_emb = np.asarray(type_emb, dtype=np.float32)
    w_l = np.asarray(w_l, dtype=np.float32)
    w_g = np.asarray(w_g, dtype=np.float32)
    b_l = float(np.asarray(b_l))
    b_g = float(np.asarray(b_g))

    nc = get_nc()
    in_maps = _host_prep(event_time, event_type, Wt_pos, type_emb,
                         w_l, b_l, w_g, b_g)
    res = run_bass_kernel_spmd(nc, in_maps, core_ids=list(range(B)))

    scores = np.stack([res.results[b]["scores_o"] for b in range(B)])
    hidden = np.stack([res.results[b]["hidden_o"] for b in range(B)])
    tdiff = np.stack([res.results[b]["tdiff_o"] for b in range(B)])
    return scores, hidden, tdiff
